# revision 73
# baseline (speedup 1.0000x reference)
"""Trainium2 Bass kernel for causal multi-head attention.

Problem: B=2, N=2048, E=1024, H=16 heads (D=64), fp32 in/out, causal,
softmax(QK^T/sqrt(D))V with four linear projections (q/k/v/o, each with bias).

Sharding over 8 NeuronCores: core = (b, g) with b in {0,1} batch, g in {0..3} a
group of 4 heads (256 of the 1024 feature channels). Each core:
  - computes Q^T/K^T (layout [f, n]) and V (layout [m, f]) for its 256 channels,
  - runs causal attention per head entirely in "transposed score" layout
    S^T[m, n] so no on-chip transposes are ever needed,
  - row sums of exp(S/8) ride along the AV matmul as an appended ones-column
    in V,
  - produces the partial output projection O_g = x_g @ w_o[:, g-slice]^T (full
    [N, E] fp16 partial).
Host sums the 4 partials per batch and adds the bias (the "all-reduce").

Numerics / math simplifications (exact up to fp16 rounding):
  - k-bias cancels in softmax (adds a per-query constant to every logit),
  - v-bias contributes exactly W_o @ b_v to the output (attention weights sum
    to one), folded into b_o on the host,
  - so only the q-bias is applied on device.
All operands are fp16 (full-rate on the PE at any moving size); accumulation
is fp32 in PSUM; softmax normalization is fp32.
"""

import sys

import numpy as np

sys.path.insert(0, "/opt/trn_rl_repo")

import concourse.bacc as bacc  # noqa: E402
import concourse.tile as tile  # noqa: E402
from concourse import mybir  # noqa: E402
from concourse.bass_utils import run_bass_kernel_spmd  # noqa: E402

B, N, E, H, D = 2, 2048, 1024, 16, 64
G = 4                       # head-groups (cores per batch)
HPG = H // G                # heads per core = 4
F = E // G                  # feature channels per core = 256
N_CORES = B * G
P = 128                     # partitions
NT = N // P                 # 16 n-tiles (and m-tiles)
ET = E // P                 # 8 e-tiles
CH = 512                    # free-dim chunk (one PSUM bank of fp32)
NCH = N // CH               # 4 aligned n-chunks
F16 = mybir.dt.float16
F32 = mybir.dt.float32

_CACHED_NC = None


def _build():
    nc = bacc.Bacc("TRN2", target_bir_lowering=False, debug=False,
                   num_devices=N_CORES)

    ht_d = nc.dram_tensor("ht", [E, N], F16, kind="ExternalInput").ap()
    wq_d = nc.dram_tensor("wq", [E, F], F16, kind="ExternalInput").ap()
    wk_d = nc.dram_tensor("wk", [E, F], F16, kind="ExternalInput").ap()
    wv_d = nc.dram_tensor("wv", [E, F], F16, kind="ExternalInput").ap()
    wo_d = nc.dram_tensor("wo", [F, E], F16, kind="ExternalInput").ap()
    bq_d = nc.dram_tensor("bq", [P, 2], F32, kind="ExternalInput").ap()
    # diagonal-tile 0/1 keep-mask: 1 where n_local >= m_local else 0
    mask_d = nc.dram_tensor("mask", [P, P], F16, kind="ExternalInput").ap()
    o_d = nc.dram_tensor("o", [N, E], F16, kind="ExternalOutput").ap()

    ht_r = ht_d.rearrange("(t p) n -> p t n", p=P)      # [128, 8, 2048]
    wq_r = wq_d.rearrange("(t p) f -> p t f", p=P)      # [128, 8, 256]
    wk_r = wk_d.rearrange("(t p) f -> p t f", p=P)
    wv_r = wv_d.rearrange("(t p) f -> p t f", p=P)
    wo_r = wo_d.rearrange("(t p) f -> p t f", p=P)      # [128, 2, 1024]

    with tile.TileContext(nc) as tc:
        with (
            tc.tile_pool(name="consts", bufs=1) as consts,
            tc.tile_pool(name="wpool", bufs=1) as wpool,
            tc.tile_pool(name="hstrip_p", bufs=4) as hstrip_p,
            tc.tile_pool(name="seq", bufs=1) as seq,
            tc.tile_pool(name="expp", bufs=14) as expp,
            tc.tile_pool(name="osb_p", bufs=4) as osb_p,
            tc.tile_pool(name="srow_p", bufs=1) as srow_p,
            tc.tile_pool(name="rrep_p", bufs=1) as rrep_p,
            tc.tile_pool(name="warm_p", bufs=1) as warm_p,
            tc.tile_pool(name="ps", bufs=2, space="PSUM") as ps_pool,
        ):
            # ---- weights first: first matmul needs only wq e-tile 0 + the
            # matching slice of strip 3 -------------------------------------
            bq_t = consts.tile([P, 2], F32, name="bq_t")
            nc.scalar.dma_start(out=bq_t, in_=bq_d)
            wq_t = expp.tile([P, ET, F], F16, name="wq_t", tag="w", bufs=3)
            nc.sync.dma_start(out=wq_t[:, 0:1, :], in_=wq_r[:, 0:1, :])
            # loaded inside strip 0, after its first use is emitted
            wk_t = expp.tile([P, ET, F], F16, name="wk_t", tag="w", bufs=3)
            wv_t = expp.tile([P, ET, F], F16, name="wv_t", tag="w", bufs=3)
            wo_t = wpool.tile([P, F // P, E], F16, name="wo_t")
            mask_t = consts.tile([P, P], F16, name="mask_t")

            # ---- PE warm-up: ~9 dummy matmuls on a memset tile ramp the
            # tensor engine to full p-state while the first DMAs land -------
            warm = warm_p.tile([P, CH], F16, name="warm")
            nc.gpsimd.memset(warm, 0.0)
            wps = ps_pool.tile([P, 2, CH], F32, name="wps", tag="big")

            def emit_warm(n):
                # dependency-free matmuls: they run whenever the PE would
                # otherwise idle on a DMA wait, keeping the p-state ramp
                # warm so the real matmuls hit full clock sooner
                for _ in range(n):
                    nc.tensor.matmul(wps[:, 0, :], warm[:, 0:P], warm,
                                     start=True, stop=True)

            emit_warm(6)

            # ---- persistent sequence-long tensors ----
            qt = [seq.tile([P, N], F16, name=f"qt{i}") for i in range(2)]
            kt = [seq.tile([P, N], F16, name=f"kt{i}") for i in range(2)]
            xt = [seq.tile([P, N], F16, name=f"xt{i}") for i in range(2)]
            vaug = seq.tile([P, NT, HPG * (D + 1)], F16, name="vaug")
            # ones column for every head slot: cols 64, 129, 194, 259
            nc.vector.memset(vaug[:, :, D::D + 1], 1.0)

            # ================= Phase 1: Q^T / K^T / V projections ========
            # Pass A computes only what heads 0,1 need (f-tile 0 of Q^T/K^T)
            # plus all of V; f-tile 1 ("pass B") is interleaved into the
            # attention steps, where ScalarE - not the PE - limits.
            _qk_half = {}

            def emit_q_group(hs, w_t, ft, s, half=None):
                # half=0/1 emits only the matching contraction half (smaller
                # PE blocks interleave with the score stream without
                # starving the exp pipeline); half=None emits both
                if half == 0 or half is None:
                    ps = ps_pool.tile([P, 2, CH], F32, name="ps_q",
                                      tag="big")
                    _qk_half[("q", ft, s)] = ps
                else:
                    ps = _qk_half.pop(("q", ft, s))
                ets = (range(ET) if half is None
                       else range(half * ET // 2, (half + 1) * ET // 2))
                for et in ets:
                    nc.tensor.matmul(
                        ps[:, 0, :],
                        w_t[:, et, ft * P:(ft + 1) * P],
                        hs[:, et, :],
                        start=(et == 0), stop=(et == ET - 1),
                    )
                if half != 0:
                    nc.vector.tensor_scalar_add(
                        out=qt[ft][:, s * CH:(s + 1) * CH],
                        in0=ps[:, 0, :],
                        scalar1=bq_t[:, ft:ft + 1],
                    )

            def emit_k_group(hs, w_t, ft, s, use_act):
                ps = ps_pool.tile([P, 2, CH], F32, name="ps_k", tag="big")
                for et in range(ET):
                    nc.tensor.matmul(
                        ps[:, 0, :],
                        w_t[:, et, ft * P:(ft + 1) * P],
                        hs[:, et, :],
                        start=(et == 0), stop=(et == ET - 1),
                    )
                dst = kt[ft][:, s * CH:(s + 1) * CH]
                if use_act:
                    nc.scalar.copy(out=dst, in_=ps[:, 0, :])
                else:
                    nc.vector.tensor_copy(out=dst, in_=ps[:, 0, :])

            first_strip = [True]
            _strips = {}
            _vstrips = {}

            def emit_strip_v(s, mi):
                # V m-tile; deferred behind the strip's score matmuls (the
                # AV consumers trail by the lookahead) so exp starts sooner
                hs = _vstrips[s]
                j = s * 4 + mi
                ps = ps_pool.tile([P, 2, CH], F32, name="ps_v", tag="big")
                for et in range(ET):
                    nc.tensor.matmul(
                        ps[:, 0, 0:F],
                        hs[:, et, mi * P:(mi + 1) * P],
                        wv_t[:, et, :],
                        start=(et == 0), stop=(et == ET - 1),
                    )
                # one strided copy covers all 4 heads (skips ones cols)
                nc.vector.tensor_copy(
                    out=vaug[:, j, :].rearrange(
                        "p (h e) -> p h e", h=HPG)[:, :, 0:D],
                    in_=ps[:, 0, 0:F].rearrange("p (h e) -> p h e", h=HPG),
                )

            def emit_strip_dma(s, split=False):
                hs = hstrip_p.tile([P, ET, CH], F16, name="hs", tag="hs")
                sl = ht_r[:, :, s * CH:(s + 1) * CH]
                if split:
                    # half-granular interleave with the wq pieces: each DMA
                    # dispatch holds the serial HWDGE device ~630ns, so
                    # fewer, bigger pieces reach the matmuls sooner
                    nc.sync.dma_start(out=hs[:, 0:4, :], in_=sl[:, 0:4, :])
                    nc.sync.dma_start(out=wq_t[:, 1:, :],
                                      in_=wq_r[:, 1:, :])
                    nc.sync.dma_start(out=hs[:, 4:, :], in_=sl[:, 4:, :])
                else:
                    nc.sync.dma_start(out=hs[:, 0:ET // 2, :],
                                      in_=sl[:, 0:ET // 2, :])
                    nc.sync.dma_start(out=hs[:, ET // 2:, :],
                                      in_=sl[:, ET // 2:, :])
                _strips[s] = hs

            def emit_strip_compute(s):
                hs = _strips.pop(s)
                first = first_strip[0]
                if first:
                    first_strip[0] = False
                    emit_q_group(hs, wq_t, 0, s)
                    # dependency-free fill: the wk/wv transfers are still in
                    # flight; keep the PE p-state ramp warm meanwhile
                    emit_warm(5)
                    # stage the non-critical loads behind the first strip's use
                    nc.scalar.dma_start(out=mask_t, in_=mask_d)
                    nc.scalar.dma_start(out=wo_t, in_=wo_r)
                    emit_k_group(hs, wk_t, 0, s, use_act=True)
                    emit_warm(4)
                else:
                    emit_q_group(hs, wq_t, 0, s)
                    emit_k_group(hs, wk_t, 0, s, use_act=True)

                _vstrips[s] = hs
                for mi in range(4):
                    emit_strip_v(s, mi)

            # pass-B work, deferred into the attention loop
            _pb_loaded = []

            def emit_passb_dma(s):
                hs2 = hstrip_p.tile([P, ET, CH], F16, name="hs", tag="hs")
                nc.sync.dma_start(out=hs2,
                                  in_=ht_r[:, :, s * CH:(s + 1) * CH])
                _pb_loaded.append((s, hs2))

            _pb_done = []

            def emit_passb_q(half):
                s, hs2 = _pb_loaded[0]
                emit_q_group(hs2, wq_t, 1, s, half=half)
                if half == 1:
                    _pb_loaded.pop(0)
                    _pb_done.append((s, hs2))

            def emit_passb_k():
                s, hs2 = _pb_done.pop(0)
                emit_k_group(hs2, wk_t, 1, s, use_act=False)

            # ================= Phase 2: attention =========================
            # Flat (head, m-tile) loop, software-pipelined: the AV matmuls of
            # step t-1 are emitted after the score matmuls of step t so the PE
            # never waits on ScalarE's exp of the tile it just produced.
            avp = {}

            def emit_scores(h, j, ex):
                qh = qt[h // 2][(h % 2) * D:(h % 2) * D + D, :]
                kh = kt[h // 2][(h % 2) * D:(h % 2) * D + D, :]
                n0 = j * P
                # 1024-aligned pieces from the diagonal; each piece is a
                # 2-bank psum tile, filled by <=512-wide matmuls (single
                # bank each) and drained by ONE exp instruction
                off = n0
                while off < N:
                    pw = min(2 * CH - (off % (2 * CH)), N - off)
                    base = off % (2 * CH)   # keep bank alignment in the tile
                    st = ps_pool.tile([P, 2, CH], F32, name="st", tag="big")
                    stv = st.rearrange("p a b -> p (a b)")
                    coff = off
                    while coff < off + pw:
                        w = min(CH - (coff % CH), off + pw - coff)
                        o0 = base + coff - off
                        nc.tensor.matmul(
                            stv[:, o0:o0 + w],
                            kh[:, n0:n0 + P], qh[:, coff:coff + w],
                            start=True, stop=True,
                        )
                        coff += w
                    nc.scalar.activation(
                        out=ex[:, off:off + pw], in_=stv[:, base:base + pw],
                        func=mybir.ActivationFunctionType.Exp,
                        scale=0.125,
                    )
                    off += pw


            def finalize(h, i0, w):
                # r = 1/rowsum for columns [i0*P, i0*P + w) (rowsum rode
                # along as V's ones column), broadcast across partitions,
                # fold into x^T. Columns < (i+1)P of an avp chunk are final
                # once m-tile i has been accumulated; later m-tiles only
                # write columns further right (subtile deps keep this
                # race-free), so a chunk can be normalized piecewise.
                c, o0 = i0 // 4, (i0 % 4) * P
                rrow = srow_p.tile([1, CH], F32, name="rrow", tag="rrow",
                                   bufs=3)[:, 0:w]
                nc.vector.reciprocal(out=rrow,
                                     in_=avp[h][D:D + 1, c, o0:o0 + w])
                rrep = rrep_p.tile([D, CH], F32, name="rrep", tag="rrep",
                                   bufs=3)[:, 0:w]
                nc.gpsimd.partition_broadcast(rrep, rrow)
                nc.vector.tensor_mul(
                    out=xt[h // 2][(h % 2) * D:(h % 2) * D + D,
                                   i0 * P:i0 * P + w],
                    in0=avp[h][0:D, c, o0:o0 + w],
                    in1=rrep,
                )

            o_items = []
            fin_q = []

            def emit_av(h, j, ex):
                c0 = (j * P) // CH
                zw = j * P - c0 * CH          # pre-diagonal columns to skip
                # post-exp causal triangle: 0/1 multiply, masked HERE (la
                # steps after the exp) rather than right after it, so the
                # DVE never head-of-line blocks on a still-running exp
                nc.vector.tensor_mul(
                    out=ex[:, j * P:(j + 1) * P],
                    in0=ex[:, j * P:(j + 1) * P], in1=mask_t)
                # diagonal chunk last: its ex also waits on the triangle mask
                for c in list(range(c0 + 1, NCH)) + [c0]:
                    if h == 0:
                        # head 0 runs m-tiles in descending strip groups
                        # (12..15, 8..11, 4..7, 0..3): chunk c's first seen
                        # contributor is j=4c, its last is j=3
                        start, stop = (j == 4 * c), (j == 3)
                    else:
                        start, stop = (j == 0), (j == 4 * c + 3)
                    lo = zw if c == c0 else 0
                    nc.tensor.matmul(
                        avp[h][:, c, lo:],
                        vaug[:, j, h * (D + 1):(h + 1) * (D + 1)],
                        ex[:, c * CH + lo:(c + 1) * CH],
                        start=start, stop=stop,
                        skip_group_check=(lo > 0),
                    )
                if h == 0:
                    if j == 3:                        # last processed m-tile
                        # stagger the four chunk finalizes across the next
                        # steps instead of bursting them at the boundary
                        fin_q.extend((0, 4 * c, CH) for c in range(NCH))
                elif h == HPG - 1 and j >= 13:
                    # last head, last chunk: finalize piecewise (columns up
                    # to tile j are complete; later m-tiles only write
                    # further right) so the final o-items unlock earlier
                    if j == 13:
                        finalize(h, 12, 2 * P)
                        o_items.extend((i, 0) for i in (12, 13))
                    elif j == 15:
                        finalize(h, 14, 2 * P)
                        o_items.extend((i, 0) for i in (14, 15))
                elif j % 4 == 3:
                    # chunk (j-3)/4 just finished: normalize it now
                    finalize(h, j - 3, CH)
                    if h == HPG - 1:
                        # all heads done for these n-tiles: output projection
                        # can stream into the remaining steps
                        o_items.extend((i, 0) for i in range(j - 3, j + 1))

            def emit_oproj(i, fc, use_act, rush=False):
                # one n-tile at a time: both 512-halves matmul into the two
                # banks of one psum slot, drained by ONE 1024-wide copy and
                # shipped as a single fully-contiguous [128, E] DMA. In rush
                # mode (the final tiles) the copy halves run on ScalarE and
                # DVE concurrently and each half ships as soon as it lands.
                ps = ps_pool.tile([P, 2, CH], F32, name="ps_o", tag="big")
                osb = osb_p.tile([P, E], F16, name="osb", tag="osb")
                for b in range(2):
                    nc.tensor.matmul(ps[:, b, :], xt[0][:, i * P:(i + 1) * P],
                                     wo_t[:, 0, b * CH:(b + 1) * CH],
                                     start=True, stop=False)
                    nc.tensor.matmul(ps[:, b, :], xt[1][:, i * P:(i + 1) * P],
                                     wo_t[:, 1, b * CH:(b + 1) * CH],
                                     start=False, stop=True)
                    if rush:
                        dst = osb[:, b * CH:(b + 1) * CH]
                        if b == 0:
                            nc.vector.tensor_copy(out=dst, in_=ps[:, b, :])
                        else:
                            nc.scalar.copy(out=dst, in_=ps[:, b, :])
                        nc.sync.dma_start(
                            out=o_d[i * P:(i + 1) * P, b * CH:(b + 1) * CH],
                            in_=dst)
                if not rush:
                    psv = ps.rearrange("p a b -> p (a b)")
                    if use_act:
                        nc.scalar.copy(out=osb, in_=psv)
                    else:
                        nc.vector.tensor_copy(out=osb, in_=psv)
                    nc.sync.dma_start(out=o_d[i * P:(i + 1) * P, :], in_=osb)

            def lookahead(h, j):
                # deeper at head starts (hides the finalize tail) and for the
                # short late m-tiles (latency-bound); shallow at the very end
                # so the last output-projection tiles free up early
                if j < 2:
                    return 7
                return 7 if j < 8 else 10

            # head 0's m-tiles ride the strip loop in descending strip groups
            # (group g's scores only need strip columns >= 4g, which are done
            # right after strip g's projections); heads 1-3 ascend normally
            h0_order = [j for g in (3, 2, 1, 0) for j in range(4 * g, 4 * g + 4)]
            steps = ([(0, j) for j in h0_order]
                     + [(h, j) for h in range(1, HPG) for j in range(NT)])
            first_of_head = {(0, h0_order[0])} | {(h, 0) for h in range(1, HPG)}

            # all pass-A DMAs queued up-front in use order (the DMA engine is
            # a serial device; keeping its queue full in consumption order
            # means the PE never waits on a feed mid-pass)
            emit_strip_dma(3, split=True)
            nc.sync.dma_start(out=wk_t, in_=wk_r)
            nc.sync.dma_start(out=wv_t, in_=wv_r)
            emit_strip_dma(2)
            emit_strip_dma(1)
            emit_strip_dma(0)
            pending = []
            n_osb = [0]
            for flat, (h, j) in enumerate(steps):
                if (h, j) in first_of_head:
                    avp[h] = ps_pool.tile([D + 1, NCH, CH], F32,
                                          name=f"av{h}", tag="av", bufs=1)
                if h == 0 and j % 4 == 0:
                    emit_strip_compute(j // 4)
                ex = expp.tile([P, N], F16, name="ex", tag="ex")
                # drain one AV step BEFORE the scores: the score matmuls
                # need a psum slot that frees while the (ready) AV work
                # runs, instead of idling at the queue head waiting for it
                if len(pending) > lookahead(h, j):
                    emit_av(*pending.pop(0))
                emit_scores(h, j, ex)
                while len(pending) > lookahead(h, j):
                    ph, pj, pex = pending.pop(0)
                    emit_av(ph, pj, pex)
                if fin_q:
                    finalize(*fin_q.pop(0))
                pending.append((h, j, ex))
                if h == 1:
                    # pass-B projections ride head 1's ScalarE-bound steps;
                    # strip DMA prefetched half a period ahead
                    if j % 4 == 0:
                        emit_passb_dma(j // 4)
                    elif j % 4 == 1:
                        emit_passb_q(0)
                    elif j % 4 == 2:
                        emit_passb_q(1)
                    elif j % 4 == 3:
                        emit_passb_k()
                if h >= 2:
                    # stream the output projection (DVE copies; ScalarE has
                    # no slack while exp runs)
                    if o_items:
                        emit_oproj(*o_items.pop(0), use_act=False)
            for ph, pj, pex in pending:
                emit_av(ph, pj, pex)
                while fin_q:
                    finalize(*fin_q.pop(0))
                for k in range(2):
                    if o_items:
                        it = o_items.pop(0)
                        emit_oproj(*it, use_act=(k == 1),
                                   rush=(it[0] >= NT - 2))

            # ================= Phase 3: output projection tail ===========
            for k, (i, fc) in enumerate(o_items):
                emit_oproj(i, fc, use_act=(k % 2 == 1),
                           rush=(i >= NT - 2))

    nc.compile()
    return nc


def _get_nc():
    global _CACHED_NC
    if _CACHED_NC is None:
        _CACHED_NC = _build()
    return _CACHED_NC


_CACHED_RUN = None


def _get_runner():
    """Cached jitted shard_map over the 8 cores (re-tracing through
    run_bass_kernel_spmd costs seconds per call; this path is ~ms)."""
    global _CACHED_RUN
    if _CACHED_RUN is not None:
        return _CACHED_RUN
    import jax
    from jax.sharding import Mesh, PartitionSpec
    from jax.experimental.shard_map import shard_map
    from concourse import bass2jax
    from concourse.bass2jax import install_neuronx_cc_hook, _bass_exec_p
    import concourse.mybir as mybir2

    nc = _get_nc()
    install_neuronx_cc_hook()
    pname = nc.partition_id_tensor.name if nc.partition_id_tensor else None
    in_names, out_names, out_avals = [], [], []
    for alloc in nc.m.functions[0].allocations:
        if not isinstance(alloc, mybir2.MemoryLocationSet):
            continue
        name = alloc.memorylocations[0].name
        if alloc.kind == "ExternalInput":
            if name != pname:
                in_names.append(name)
        elif alloc.kind == "ExternalOutput":
            out_names.append(name)
            out_avals.append(jax.core.ShapedArray(
                tuple(alloc.tensor_shape), mybir.dt.np(alloc.dtype)))
    n_params = len(in_names)
    all_in = list(in_names) + list(out_names)
    if pname:
        all_in.append(pname)

    def _body(*args):
        operands = list(args)
        if pname is not None:
            operands.append(bass2jax.partition_id_tensor())
        outs = _bass_exec_p.bind(
            *operands, out_avals=tuple(out_avals), in_names=tuple(all_in),
            out_names=tuple(out_names), lowering_input_output_aliases=(),
            sim_require_finite=True, sim_require_nnan=True, nc=nc)
        return tuple(outs)

    devices = jax.devices()[:N_CORES]
    mesh = Mesh(np.asarray(devices), ("core",))
    n_outs = len(out_avals)
    fn = jax.jit(
        shard_map(_body, mesh=mesh,
                  in_specs=(PartitionSpec("core"),) * (n_params + n_outs),
                  out_specs=(PartitionSpec("core"),) * n_outs,
                  check_rep=False),
        keep_unused=True)

    def run(in_maps):
        concat_in = [np.concatenate([np.asarray(in_maps[c][nm])
                                     for c in range(N_CORES)], axis=0)
                     for nm in in_names]
        concat_zeros = [np.zeros((N_CORES * a.shape[0], *a.shape[1:]),
                                 a.dtype) for a in out_avals]
        outs = fn(*concat_in, *concat_zeros)
        return [{nm: np.asarray(outs[i]).reshape(N_CORES, *out_avals[i].shape)[c]
                 for i, nm in enumerate(out_names)} for c in range(N_CORES)]

    _CACHED_RUN = run
    return run


def _make_in_maps(hidden_state, w_q, b_q, w_k, w_v, w_o):
    mask = (np.arange(P)[None, :] >= np.arange(P)[:, None]).astype(np.float16)
    hT = [np.ascontiguousarray(hidden_state[b].T).astype(np.float16)
          for b in range(B)]
    in_maps = []
    for b in range(B):
        for g in range(G):
            sl = slice(g * F, (g + 1) * F)
            bq = np.stack([b_q[sl][:P], b_q[sl][P:]], axis=1)
            in_maps.append({
                "ht": hT[b],
                "wq": np.ascontiguousarray(w_q[sl, :].T).astype(np.float16),
                "wk": np.ascontiguousarray(w_k[sl, :].T).astype(np.float16),
                "wv": np.ascontiguousarray(w_v[sl, :].T).astype(np.float16),
                "wo": np.ascontiguousarray(w_o[:, sl].T).astype(np.float16),
                "bq": bq.astype(np.float32),
                "mask": mask,
            })
    return in_maps


def kernel(hidden_state, w_q, b_q, w_k, b_k, w_v, b_v, w_o, b_o, **run_kwargs):
    hidden_state = np.asarray(hidden_state, dtype=np.float32)
    w_q = np.asarray(w_q, dtype=np.float32)
    b_q = np.asarray(b_q, dtype=np.float32)
    w_k = np.asarray(w_k, dtype=np.float32)
    w_v = np.asarray(w_v, dtype=np.float32)
    b_v = np.asarray(b_v, dtype=np.float32)
    w_o = np.asarray(w_o, dtype=np.float32)
    b_o = np.asarray(b_o, dtype=np.float32)
    # v-bias folds into the output bias: attention weights sum to one, so
    # x = attn @ (v + b_v) = attn @ v + b_v, and (x + b_v) @ W_o^T + b_o
    # = x @ W_o^T + (b_o + W_o b_v). k-bias cancels in the softmax.
    b_o_eff = b_o + w_o @ b_v

    in_maps = _make_in_maps(hidden_state, w_q, b_q, w_k, w_v, w_o)
    if run_kwargs:
        res = run_bass_kernel_spmd(_get_nc(), in_maps,
                                   core_ids=list(range(N_CORES)), **run_kwargs)
        kernel.last_result = res
        results = res.results
    else:
        results = _get_runner()(in_maps)
    out = np.empty((B, N, E), dtype=np.float32)
    for b in range(B):
        acc = results[b * G]["o"].astype(np.float32)
        for g in range(1, G):
            acc = acc + results[b * G + g]["o"].astype(np.float32)
        out[b] = acc + b_o_eff[None, :]
    return out
